# revision 8
# baseline (speedup 1.0000x reference)
"""MoE (top-2 of 8 experts, d=1024) — expert-parallel Bass kernel for 8 trn2 cores.

Strategy (per sharding_hint "Expert-parallel"): shard W1/W2/b1/b2 along the
expert axis (expert e -> core e). The host computes the gate scores and top-2
assignment (0.2% of model FLOPs, deterministic) to build the dispatch: each
core receives exactly the tokens routed to its expert (padded to capacity C,
transposed and chunk-major so every HBM block streams sequentially). Each core
computes   yT = (relu(W1^T xT + b1)^T W2 + b2) * w   with float32r matmuls
(full PE rate, ~1e-4 accuracy); the host scatter-adds the two expert
contributions per token (the "combine" of the return all-to-all).

Device-side details:
 - per-kc split DMAs so the first matmul waits on 0.75MB, not 10MB
 - combine weights broadcast across partitions on-device (K=1 ones matmul)
 - chunk sizes [512]*nb + [tail] with C a multiple of 256 (f32r keeps
   1 cycle/row down to a free dim of 256)
"""

import numpy as np

import concourse.bass as bass
import concourse.mybir as mybir
import concourse.tile as tile
from concourse import bacc
from concourse.bass_utils import run_bass_kernel_spmd

# Problem shapes (hardcoded per contract)
D = 1024  # d_model == d_hidden
N_EXPERTS = 8
TOP_K = 2
N_CORES = 8
B, T = 4, 2048
N_TOKENS = B * T

F32 = mybir.dt.float32
F32R = mybir.dt.float32r
KC = D // 128  # contraction chunks (8)
MC = D // 128  # output-feature chunks (8)
NT = 512      # tokens per matmul (moving free dim; fp32 max)
CGRAIN = 256  # capacity granularity (f32r needs free dim >= 256 for full rate)


def chunk_sizes(C):
    assert C % CGRAIN == 0
    sizes = [NT] * (C // NT)
    if C % NT:
        sizes.append(C % NT)
    return sizes


def build_moe_expert_kernel(C: int, repeat: int = 1) -> bacc.Bacc:
    """One-expert MLP kernel: yT = (relu(x@W1+b1)@W2 + b2) * w, chunk-major.

    DRAM inputs: xTb [nb, D, NT] (+ xTt [D, tail] if C%NT), wvec [1, C],
    ones [1, 128], w1 [D, D], b1 [D], w2 [D, D], b2 [D].
    Outputs: yTb [nb, D, NT] (+ yTt [D, tail]).
    `repeat` wraps the computation in a hardware loop (slope-based HW timing).
    """
    sizes = chunk_sizes(C)
    nb = sum(1 for s in sizes if s == NT)
    tail = C % NT

    nc = bacc.Bacc("TRN2", target_bir_lowering=False, debug=False,
                   num_devices=N_CORES)

    xTb = nc.dram_tensor("xTb", [nb, D, NT], F32R, kind="ExternalInput")
    wvec = nc.dram_tensor("wvec", [1, C], F32R, kind="ExternalInput")
    ones = nc.dram_tensor("ones", [1, 128], F32R, kind="ExternalInput")
    w1 = nc.dram_tensor("w1", [D, D], F32R, kind="ExternalInput")
    b1 = nc.dram_tensor("b1", [D], F32, kind="ExternalInput")
    w2 = nc.dram_tensor("w2", [D, D], F32R, kind="ExternalInput")
    b2 = nc.dram_tensor("b2", [D], F32, kind="ExternalInput")
    yTb = nc.dram_tensor("yTb", [nb, D, NT], F32, kind="ExternalOutput")
    if tail:
        xTt = nc.dram_tensor("xTt", [D, tail], F32R, kind="ExternalInput")
        yTt = nc.dram_tensor("yTt", [D, tail], F32, kind="ExternalOutput")

    # DRAM views: partition-dim-first tilings (chunk blocks are contiguous)
    xTb_v = xTb.ap().rearrange("n (kc kp) t -> n kp kc t", kc=KC)
    w1_v = w1.ap().rearrange("(kc kp) m -> kp kc m", kc=KC)      # [128, KC, D]
    w2_v = w2.ap().rearrange("(kc kp) m -> kp kc m", kc=KC)
    b1_v = b1.ap().rearrange("(mc mp) -> mp mc", mc=MC)          # [128, MC]
    b2_v = b2.ap().rearrange("(mc mp) -> mp mc", mc=MC)
    yTb_v = yTb.ap().rearrange("n (mc mp) t -> n mp mc t", mc=MC)
    if tail:
        xTt_v = xTt.ap().rearrange("(kc kp) t -> kp kc t", kc=KC)
        yTt_v = yTt.ap().rearrange("(mc mp) t -> mp mc t", mc=MC)

    def x_view(n):
        return xTb_v[n] if sizes[n] == NT else xTt_v

    def y_view(n):
        return yTb_v[n] if sizes[n] == NT else yTt_v

    with tile.TileContext(nc) as tc:
        with (
            tc.tile_pool(name="weights", bufs=1) as wpool,
            tc.tile_pool(name="consts", bufs=1) as cpool,
            tc.tile_pool(name="xin", bufs=3) as xpool,
            tc.tile_pool(name="hmid", bufs=2) as hpool,
            tc.tile_pool(name="yout", bufs=2) as ypool,
            tc.tile_pool(name="wbp", bufs=2) as wbpool,
            tc.tile_pool(name="ph", bufs=3, space="PSUM") as phpool,
            tc.tile_pool(name="py", bufs=3, space="PSUM") as pypool,
            tc.tile_pool(name="pw", bufs=2, space="PSUM") as pwpool,
        ):
            from contextlib import nullcontext
            loop_cm = (
                tc.For_i(0, repeat, 1,
                         hint_engines=(mybir.EngineType.PE,
                                       mybir.EngineType.Activation,
                                       mybir.EngineType.DVE,
                                       mybir.EngineType.SP))
                if repeat > 1 else nullcontext()
            )
            with loop_cm:
                # Per-kc split DMAs: the first matmul only waits for its own
                # 512KB weight slice + 256KB x slice instead of the whole
                # prologue (model: first MM 36.6us -> 5.1us).
                w1_sb = wpool.tile([128, KC, D], F32R, tag="w1")
                w2_sb = wpool.tile([128, KC, D], F32R, tag="w2")
                b1_sb = cpool.tile([128, MC], F32, tag="b1")
                b2_sb = cpool.tile([128, MC], F32, tag="b2")
                wv_sb = cpool.tile([1, C], F32R, tag="wv")
                on_sb = cpool.tile([1, 128], F32R, tag="ones")
                x0 = xpool.tile([128, KC, NT], F32R, tag="x")
                for kc in range(KC):
                    nc.sync.dma_start(w1_sb[:, kc, :], w1_v[:, kc, :])
                    nc.sync.dma_start(x0[:, kc, :sizes[0]], x_view(0)[:, kc, :])
                nc.sync.dma_start(b1_sb[:], b1_v)
                nc.sync.dma_start(on_sb[:], ones.ap())
                nc.sync.dma_start(wv_sb[:], wvec.ap())
                for kc in range(KC):
                    nc.sync.dma_start(w2_sb[:, kc, :], w2_v[:, kc, :])
                nc.sync.dma_start(b2_sb[:], b2_v)

                coff = 0
                for n, sz in enumerate(sizes):
                    if n == 0:
                        x_sb = x0
                    else:
                        x_sb = xpool.tile([128, KC, NT], F32R, tag="x")
                        for kc in range(KC):
                            nc.sync.dma_start(x_sb[:, kc, :sz], x_view(n)[:, kc, :])

                    # broadcast combine weights for this chunk: [128, sz]
                    pw = pwpool.tile([128, NT], F32, tag="pw")
                    nc.tensor.matmul(pw[:, :sz], on_sb[:],
                                     wv_sb[:, coff:coff + sz],
                                     start=True, stop=True)
                    wb_sb = wbpool.tile([128, NT], F32, tag="wb")
                    nc.vector.tensor_copy(wb_sb[:, :sz], pw[:, :sz])

                    h_sb = hpool.tile([128, KC, NT], F32R, tag="h")
                    for mc in range(MC):
                        ph = phpool.tile([128, NT], F32, tag="ph")
                        for kc in range(KC):
                            nc.tensor.matmul(
                                ph[:, :sz],
                                w1_sb[:, kc, bass.ts(mc, 128)],
                                x_sb[:, kc, :sz],
                                start=(kc == 0), stop=(kc == KC - 1),
                            )
                        # h = relu(ph + b1)
                        nc.scalar.activation(
                            h_sb[:, mc, :sz], ph[:, :sz],
                            mybir.ActivationFunctionType.Relu,
                            bias=b1_sb[:, mc:mc + 1],
                        )

                    y_sb = ypool.tile([128, MC, NT], F32, tag="y")
                    for mc in range(MC):
                        py = pypool.tile([128, NT], F32, tag="py")
                        for kc in range(KC):
                            nc.tensor.matmul(
                                py[:, :sz],
                                w2_sb[:, kc, bass.ts(mc, 128)],
                                h_sb[:, kc, :sz],
                                start=(kc == 0), stop=(kc == KC - 1),
                            )
                        # y = (py + b2) * w
                        nc.scalar.activation(
                            y_sb[:, mc, :sz], py[:, :sz],
                            mybir.ActivationFunctionType.Identity,
                            bias=b2_sb[:, mc:mc + 1],
                        )
                        nc.vector.tensor_mul(
                            y_sb[:, mc, :sz], y_sb[:, mc, :sz], wb_sb[:, :sz],
                        )
                        nc.sync.dma_start(y_view(n)[:, mc, :], y_sb[:, mc, :sz])
                    coff += sz

    nc.compile()
    return nc


_NC_CACHE: dict = {}


def _get_kernel(C: int, repeat: int = 1) -> bacc.Bacc:
    key = (C, repeat)
    if key not in _NC_CACHE:
        _NC_CACHE[key] = build_moe_expert_kernel(C, repeat)
    return _NC_CACHE[key]


def dispatch(x, W_gate, b_gate):
    """Host-side gate + top-2 dispatch plan. Returns (xf, ids, wts, C)."""
    xf = np.ascontiguousarray(np.asarray(x).reshape(-1, D), dtype=np.float32)
    scores = xf @ np.asarray(W_gate, np.float32) + np.asarray(b_gate, np.float32)
    # top-2 expert ids per token (order irrelevant: contributions are summed)
    top2 = np.argpartition(scores, N_EXPERTS - TOP_K, axis=1)[:, -TOP_K:]
    ids, wts = [], []
    for e in range(N_EXPERTS):
        tok = np.nonzero((top2 == e).any(axis=1))[0]
        ids.append(tok)
        wts.append(scores[tok, e])
    max_cnt = max(len(t) for t in ids)
    C = ((max_cnt + CGRAIN - 1) // CGRAIN) * CGRAIN
    return xf, ids, wts, C


def make_in_maps(parts, xf, ids, wts, C):
    """Build per-core input dicts (chunk-major xT blocks)."""
    W1, b1, W2, b2 = parts
    sizes = chunk_sizes(C)
    nb = sum(1 for s in sizes if s == NT)
    tail = C % NT
    in_maps = []
    for e in range(N_EXPERTS):
        cnt = len(ids[e])
        xTe = np.zeros((D, C), np.float32)
        xTe[:, :cnt] = xf[ids[e]].T
        xb = np.ascontiguousarray(
            xTe[:, :nb * NT].reshape(D, nb, NT).transpose(1, 0, 2))
        wv = np.zeros((1, C), np.float32)
        wv[0, :cnt] = wts[e]
        m = {
            "xTb": xb, "wvec": wv,
            "ones": np.ones((1, 128), np.float32),
            "w1": np.ascontiguousarray(W1[e]), "b1": b1[e],
            "w2": np.ascontiguousarray(W2[e]), "b2": b2[e],
        }
        if tail:
            m["xTt"] = np.ascontiguousarray(xTe[:, nb * NT:])
        in_maps.append(m)
    return in_maps


def kernel(x, W_gate, b_gate, W1, b1, W2, b2):
    xf, ids, wts, C = dispatch(x, W_gate, b_gate)
    nc = _get_kernel(C)

    W1 = np.asarray(W1, np.float32)
    W2 = np.asarray(W2, np.float32)
    b1 = np.asarray(b1, np.float32)
    b2 = np.asarray(b2, np.float32)
    in_maps = make_in_maps((W1, b1, W2, b2), xf, ids, wts, C)

    res = run_bass_kernel_spmd(nc, in_maps, core_ids=list(range(N_CORES)))

    sizes = chunk_sizes(C)
    nb = sum(1 for s in sizes if s == NT)
    tail = C % NT
    out = np.zeros((N_TOKENS, D), np.float32)
    for e in range(N_EXPERTS):
        cnt = len(ids[e])
        r = res.results[e]
        yTe = r["yTb"].transpose(1, 0, 2).reshape(D, nb * NT)
        if tail:
            yTe = np.concatenate([yTe, r["yTt"]], axis=1)
        out[ids[e]] += yTe.T[:cnt]
    return out.reshape(B, T, D)


# revision 15
# speedup vs baseline: 1.8113x; 1.8113x over previous
"""MoE (top-2 of 8 experts, d=1024) — expert-parallel Bass kernel for 8 trn2 cores.

Strategy (per sharding_hint "Expert-parallel"): shard W1/W2/b1/b2 along the
expert axis (expert e -> core e). The host computes the gate scores and top-2
assignment (0.2% of model FLOPs, deterministic) to build the dispatch: each
core receives exactly the tokens routed to its expert (padded to capacity C,
transposed and chunk-major so every HBM block streams sequentially). Each core
computes   yT = (relu(W1^T xT + b1)^T W2 + b2) * w   with float32r matmuls
(full PE rate, ~1e-4 accuracy); the host scatter-adds the two expert
contributions per token (the "combine" of the return all-to-all).

Device-side details:
 - per-kc split DMAs so the first matmul waits on 0.75MB, not 10MB
 - combine weights broadcast across partitions on-device (K=1 ones matmul,
   all chunks upfront) so no per-token broadcast traffic from HBM
 - bias-add + relu fused into single DVE tensor_scalar ops reading PSUM
   (keeps ScalarE off the critical path)
 - chunk sizes [512]*nb + [tail] with C a multiple of 256 (f32r keeps
   1 cycle/row down to a free dim of 256)
"""

import numpy as np

import concourse.bass as bass
import concourse.mybir as mybir
import concourse.tile as tile
from concourse import bacc
from concourse.bass_utils import run_bass_kernel_spmd

# Problem shapes (hardcoded per contract)
D = 1024  # d_model == d_hidden
N_EXPERTS = 8
TOP_K = 2
N_CORES = 8
B, T = 4, 2048
N_TOKENS = B * T

F32 = mybir.dt.float32
F32R = mybir.dt.float32r
KC = D // 128  # contraction chunks (8)
MC = D // 128  # output-feature chunks (8)
NT = 512      # tokens per matmul (moving free dim; fp32 max)
CGRAIN = 256  # capacity granularity (f32r needs free dim >= 256 for full rate)


def chunk_sizes(C):
    assert C % CGRAIN == 0
    sizes = [NT] * (C // NT)
    if C % NT:
        sizes.append(C % NT)
    return sizes


def build_moe_expert_kernel(C: int, repeat: int = 1, split_w: int = 8,
                            split_x: bool = True, split_y: bool = True,
                            pipe: bool = False, stagger: bool = False,
                            dve_elt: bool = True,
                            wb_all: bool = True) -> bacc.Bacc:
    """One-expert MLP kernel: yT = (relu(x@W1+b1)@W2 + b2) * w, chunk-major.

    DRAM inputs: xTb [nb, D, NT] (+ xTt [D, tail] if C%NT), wvec [1, C],
    ones [1, 128], w1 [D, D], b1 [D], w2 [D, D], b2 [D].
    Outputs: yTb [nb, D, NT] (+ yTt [D, tail]).
    `repeat` wraps the computation in a hardware loop (slope-based HW timing).
    """
    sizes = chunk_sizes(C)
    nb = sum(1 for s in sizes if s == NT)
    tail = C % NT

    nc = bacc.Bacc("TRN2", target_bir_lowering=False, debug=False,
                   num_devices=N_CORES)

    xTb = nc.dram_tensor("xTb", [nb, D, NT], F32R, kind="ExternalInput")
    wvec = nc.dram_tensor("wvec", [1, C], F32R, kind="ExternalInput")
    ones = nc.dram_tensor("ones", [1, 128], F32R, kind="ExternalInput")
    w1 = nc.dram_tensor("w1", [D, D], F32R, kind="ExternalInput")
    b1 = nc.dram_tensor("b1", [D], F32, kind="ExternalInput")
    w2 = nc.dram_tensor("w2", [D, D], F32R, kind="ExternalInput")
    b2 = nc.dram_tensor("b2", [D], F32, kind="ExternalInput")
    yTb = nc.dram_tensor("yTb", [nb, D, NT], F32, kind="ExternalOutput")
    if tail:
        xTt = nc.dram_tensor("xTt", [D, tail], F32R, kind="ExternalInput")
        yTt = nc.dram_tensor("yTt", [D, tail], F32, kind="ExternalOutput")

    # DRAM views: partition-dim-first tilings (chunk blocks are contiguous)
    xTb_v = xTb.ap().rearrange("n (kc kp) t -> n kp kc t", kc=KC)
    w1_v = w1.ap().rearrange("(kc kp) m -> kp kc m", kc=KC)      # [128, KC, D]
    w2_v = w2.ap().rearrange("(kc kp) m -> kp kc m", kc=KC)
    b1_v = b1.ap().rearrange("(mc mp) -> mp mc", mc=MC)          # [128, MC]
    b2_v = b2.ap().rearrange("(mc mp) -> mp mc", mc=MC)
    yTb_v = yTb.ap().rearrange("n (mc mp) t -> n mp mc t", mc=MC)
    if tail:
        xTt_v = xTt.ap().rearrange("(kc kp) t -> kp kc t", kc=KC)
        yTt_v = yTt.ap().rearrange("(mc mp) t -> mp mc t", mc=MC)

    def x_view(n):
        return xTb_v[n] if sizes[n] == NT else xTt_v

    def y_view(n):
        return yTb_v[n] if sizes[n] == NT else yTt_v

    with tile.TileContext(nc) as tc:
        with (
            tc.tile_pool(name="weights", bufs=1) as wpool,
            tc.tile_pool(name="consts", bufs=1) as cpool,
            tc.tile_pool(name="xin", bufs=3) as xpool,
            tc.tile_pool(name="hmid", bufs=2) as hpool,
            tc.tile_pool(name="yout", bufs=2) as ypool,
            tc.tile_pool(name="wbp", bufs=2) as wbpool,
            tc.tile_pool(name="ph", bufs=3, space="PSUM") as phpool,
            tc.tile_pool(name="py", bufs=3, space="PSUM") as pypool,
            tc.tile_pool(name="pw", bufs=2, space="PSUM") as pwpool,
        ):
            from contextlib import nullcontext
            loop_cm = (
                tc.For_i(0, repeat, 1,
                         hint_engines=(mybir.EngineType.PE,
                                       mybir.EngineType.Activation,
                                       mybir.EngineType.DVE,
                                       mybir.EngineType.SP),
                         staggered_reset=stagger)
                if repeat > 1 else nullcontext()
            )
            with loop_cm:
                # Per-kc split DMAs: the first matmul only waits for its own
                # 512KB weight slice + 256KB x slice instead of the whole
                # prologue (model: first MM 36.6us -> 5.1us).
                w1_sb = wpool.tile([128, KC, D], F32R, tag="w1")
                w2_sb = wpool.tile([128, KC, D], F32R, tag="w2")
                b1_sb = cpool.tile([128, MC], F32, tag="b1")
                b2_sb = cpool.tile([128, MC], F32, tag="b2")
                wv_sb = cpool.tile([1, C], F32R, tag="wv")
                on_sb = cpool.tile([1, 128], F32R, tag="ones")
                x0 = xpool.tile([128, KC, NT], F32R, tag="x")
                g = KC // split_w
                for i in range(split_w):
                    ks = slice(i * g, (i + 1) * g)
                    nc.sync.dma_start(w1_sb[:, ks, :], w1_v[:, ks, :])
                    if split_x:
                        for kc in range(i * g, (i + 1) * g):
                            nc.sync.dma_start(x0[:, kc, :sizes[0]],
                                              x_view(0)[:, kc, :])
                if not split_x:
                    nc.sync.dma_start(x0[:, :, :sizes[0]], x_view(0))
                nc.sync.dma_start(b1_sb[:], b1_v)
                nc.sync.dma_start(on_sb[:], ones.ap())
                nc.sync.dma_start(wv_sb[:], wvec.ap())
                for i in range(split_w):
                    ks = slice(i * g, (i + 1) * g)
                    nc.sync.dma_start(w2_sb[:, ks, :], w2_v[:, ks, :])
                nc.sync.dma_start(b2_sb[:], b2_v)

                offs = [sum(sizes[:i]) for i in range(len(sizes))]
                nchk = len(sizes)

                wb_full = None
                if wb_all:
                    wb_full = cpool.tile([128, C], F32, tag="wbf")
                    for n in range(nchk):
                        pwf = pwpool.tile([128, NT], F32, tag="pw")
                        nc.tensor.matmul(pwf[:, :sizes[n]], on_sb[:],
                                         wv_sb[:, offs[n]:offs[n] + sizes[n]],
                                         start=True, stop=True)
                        nc.vector.tensor_copy(
                            wb_full[:, offs[n]:offs[n] + sizes[n]],
                            pwf[:, :sizes[n]])

                def emit_x_dma(n, x_sb):
                    sz = sizes[n]
                    if split_x:
                        for kc in range(KC):
                            nc.sync.dma_start(x_sb[:, kc, :sz],
                                              x_view(n)[:, kc, :])
                    else:
                        nc.sync.dma_start(x_sb[:, :, :sz], x_view(n))

                def emit_wb(n):
                    # broadcast combine weights for chunk n: [128, sz]
                    if wb_all:
                        return wb_full[:, offs[n]:offs[n] + sizes[n]]
                    sz = sizes[n]
                    pw = pwpool.tile([128, NT], F32, tag="pw")
                    nc.tensor.matmul(pw[:, :sz], on_sb[:],
                                     wv_sb[:, offs[n]:offs[n] + sz],
                                     start=True, stop=True)
                    wb_sb = wbpool.tile([128, NT], F32, tag="wb")
                    nc.vector.tensor_copy(wb_sb[:, :sz], pw[:, :sz])
                    return wb_sb

                def emit_h_mc(n, mc, x_sb, h_sb):
                    sz = sizes[n]
                    ph = phpool.tile([128, NT], F32, tag="ph")
                    for kc in range(KC):
                        nc.tensor.matmul(
                            ph[:, :sz],
                            w1_sb[:, kc, bass.ts(mc, 128)],
                            x_sb[:, kc, :sz],
                            start=(kc == 0), stop=(kc == KC - 1),
                        )
                    # h = relu(ph + b1)
                    if dve_elt:
                        nc.vector.tensor_scalar(
                            h_sb[:, mc, :sz], ph[:, :sz],
                            b1_sb[:, mc:mc + 1], 0.0,
                            mybir.AluOpType.add, mybir.AluOpType.max,
                        )
                    else:
                        nc.scalar.activation(
                            h_sb[:, mc, :sz], ph[:, :sz],
                            mybir.ActivationFunctionType.Relu,
                            bias=b1_sb[:, mc:mc + 1],
                        )

                def emit_y_mc(n, mc, h_sb, y_sb, wb_sb):
                    sz = sizes[n]
                    py = pypool.tile([128, NT], F32, tag="py")
                    for kc in range(KC):
                        nc.tensor.matmul(
                            py[:, :sz],
                            w2_sb[:, kc, bass.ts(mc, 128)],
                            h_sb[:, kc, :sz],
                            start=(kc == 0), stop=(kc == KC - 1),
                        )
                    # y = (py + b2) * w
                    if dve_elt:
                        nc.vector.tensor_scalar(
                            y_sb[:, mc, :sz], py[:, :sz],
                            b2_sb[:, mc:mc + 1], None,
                            mybir.AluOpType.add,
                        )
                    else:
                        nc.scalar.activation(
                            y_sb[:, mc, :sz], py[:, :sz],
                            mybir.ActivationFunctionType.Identity,
                            bias=b2_sb[:, mc:mc + 1],
                        )
                    nc.vector.tensor_mul(
                        y_sb[:, mc, :sz], y_sb[:, mc, :sz],
                        wb_sb if wb_all else wb_sb[:, :sz],
                    )
                    if split_y:
                        nc.sync.dma_start(y_view(n)[:, mc, :],
                                          y_sb[:, mc, :sz])

                if not pipe:
                    for n in range(nchk):
                        if n == 0:
                            x_sb = x0
                        else:
                            x_sb = xpool.tile([128, KC, NT], F32R, tag="x")
                            emit_x_dma(n, x_sb)
                        wb_sb = emit_wb(n)
                        h_sb = hpool.tile([128, KC, NT], F32R, tag="h")
                        for mc in range(MC):
                            emit_h_mc(n, mc, x_sb, h_sb)
                        y_sb = ypool.tile([128, MC, NT], F32, tag="y")
                        for mc in range(MC):
                            emit_y_mc(n, mc, h_sb, y_sb, wb_sb)
                        if not split_y:
                            nc.sync.dma_start(y_view(n), y_sb[:, :, :sizes[n]])
                else:
                    # software pipeline: stage s emits h-phase(s) interleaved
                    # with y-phase(s-1) at mc granularity, so the PE never
                    # waits on the relu tail of a chunk before starting the
                    # next chunk's first-layer matmuls.
                    x_tiles = {0: x0}
                    h_tiles = {}
                    y_tiles = {}
                    wb_tiles = {0: emit_wb(0)}
                    for s in range(nchk + 1):
                        if s + 1 < nchk:  # prefetch x for next stage
                            xt = xpool.tile([128, KC, NT], F32R, tag="x")
                            emit_x_dma(s + 1, xt)
                            x_tiles[s + 1] = xt
                        if s < nchk:
                            h_tiles[s] = hpool.tile([128, KC, NT], F32R, tag="h", name=f"hs{s}")
                            if s + 1 < nchk:
                                wb_tiles[s + 1] = emit_wb(s + 1)
                        if s > 0:
                            y_tiles[s - 1] = ypool.tile([128, MC, NT], F32, tag="y", name=f"ys{s}")
                        for mc in range(MC):
                            if s < nchk:
                                emit_h_mc(s, mc, x_tiles[s], h_tiles[s])
                            if s > 0:
                                emit_y_mc(s - 1, mc, h_tiles[s - 1],
                                          y_tiles[s - 1], wb_tiles[s - 1])
                        if s > 0 and not split_y:
                            nc.sync.dma_start(y_view(s - 1),
                                              y_tiles[s - 1][:, :, :sizes[s - 1]])
                        x_tiles.pop(s - 1, None)

    nc.compile()
    return nc


_NC_CACHE: dict = {}


def _get_kernel(C: int, repeat: int = 1, **opts) -> bacc.Bacc:
    key = (C, repeat, tuple(sorted(opts.items())))
    if key not in _NC_CACHE:
        _NC_CACHE[key] = build_moe_expert_kernel(C, repeat, **opts)
    return _NC_CACHE[key]


def dispatch(x, W_gate, b_gate):
    """Host-side gate + top-2 dispatch plan. Returns (xf, ids, wts, C)."""
    xf = np.ascontiguousarray(np.asarray(x).reshape(-1, D), dtype=np.float32)
    scores = xf @ np.asarray(W_gate, np.float32) + np.asarray(b_gate, np.float32)
    # top-2 expert ids per token (order irrelevant: contributions are summed)
    top2 = np.argpartition(scores, N_EXPERTS - TOP_K, axis=1)[:, -TOP_K:]
    ids, wts = [], []
    for e in range(N_EXPERTS):
        tok = np.nonzero((top2 == e).any(axis=1))[0]
        ids.append(tok)
        wts.append(scores[tok, e])
    max_cnt = max(len(t) for t in ids)
    C = ((max_cnt + CGRAIN - 1) // CGRAIN) * CGRAIN
    return xf, ids, wts, C


def make_in_maps(parts, xf, ids, wts, C):
    """Build per-core input dicts (chunk-major xT blocks)."""
    W1, b1, W2, b2 = parts
    sizes = chunk_sizes(C)
    nb = sum(1 for s in sizes if s == NT)
    tail = C % NT
    in_maps = []
    for e in range(N_EXPERTS):
        cnt = len(ids[e])
        xTe = np.zeros((D, C), np.float32)
        xTe[:, :cnt] = xf[ids[e]].T
        xb = np.ascontiguousarray(
            xTe[:, :nb * NT].reshape(D, nb, NT).transpose(1, 0, 2))
        wv = np.zeros((1, C), np.float32)
        wv[0, :cnt] = wts[e]
        m = {
            "xTb": xb, "wvec": wv,
            "ones": np.ones((1, 128), np.float32),
            "w1": np.ascontiguousarray(W1[e]), "b1": b1[e],
            "w2": np.ascontiguousarray(W2[e]), "b2": b2[e],
        }
        if tail:
            m["xTt"] = np.ascontiguousarray(xTe[:, nb * NT:])
        in_maps.append(m)
    return in_maps


def kernel(x, W_gate, b_gate, W1, b1, W2, b2):
    xf, ids, wts, C = dispatch(x, W_gate, b_gate)
    nc = _get_kernel(C)

    W1 = np.asarray(W1, np.float32)
    W2 = np.asarray(W2, np.float32)
    b1 = np.asarray(b1, np.float32)
    b2 = np.asarray(b2, np.float32)
    in_maps = make_in_maps((W1, b1, W2, b2), xf, ids, wts, C)

    res = run_bass_kernel_spmd(nc, in_maps, core_ids=list(range(N_CORES)))

    sizes = chunk_sizes(C)
    nb = sum(1 for s in sizes if s == NT)
    tail = C % NT
    out = np.zeros((N_TOKENS, D), np.float32)
    for e in range(N_EXPERTS):
        cnt = len(ids[e])
        r = res.results[e]
        yTe = r["yTb"].transpose(1, 0, 2).reshape(D, nb * NT)
        if tail:
            yTe = np.concatenate([yTe, r["yTt"]], axis=1)
        out[ids[e]] += yTe.T[:cnt]
    return out.reshape(B, T, D)


# revision 16
# speedup vs baseline: 1.8738x; 1.0345x over previous
"""MoE (top-2 of 8 experts, d=1024) — expert-parallel Bass kernel for 8 trn2 cores.

Strategy (per sharding_hint "Expert-parallel"): shard W1/W2/b1/b2 along the
expert axis (expert e -> core e). The host computes the gate scores and top-2
assignment (0.2% of model FLOPs, deterministic) to build the dispatch: each
core receives exactly the tokens routed to its expert (padded to capacity C,
transposed and chunk-major so every HBM block streams sequentially). Each core
computes   yT = (relu(W1^T xT + b1)^T W2 + b2) * w   with float32r matmuls
(full PE rate, ~1e-4 accuracy); the host scatter-adds the two expert
contributions per token (the "combine" of the return all-to-all).

Device-side details:
 - per-kc split DMAs so the first matmul waits on 0.75MB, not 10MB
 - combine weights broadcast across partitions on-device (K=1 ones matmul,
   all chunks upfront) so no per-token broadcast traffic from HBM
 - bias-add + relu fused into single DVE tensor_scalar ops reading PSUM
   (keeps ScalarE off the critical path)
 - chunk sizes [512]*nb + [tail] with C a multiple of 256 (f32r keeps
   1 cycle/row down to a free dim of 256)
"""

import numpy as np

import concourse.bass as bass
import concourse.mybir as mybir
import concourse.tile as tile
from concourse import bacc
from concourse.bass_utils import run_bass_kernel_spmd

# Problem shapes (hardcoded per contract)
D = 1024  # d_model == d_hidden
N_EXPERTS = 8
TOP_K = 2
N_CORES = 8
B, T = 4, 2048
N_TOKENS = B * T

F32 = mybir.dt.float32
F32R = mybir.dt.float32r
KC = D // 128  # contraction chunks (8)
MC = D // 128  # output-feature chunks (8)
NT = 512      # tokens per matmul (moving free dim; fp32 max)
CGRAIN = 256  # capacity granularity (f32r needs free dim >= 256 for full rate)


def chunk_sizes(C):
    assert C % CGRAIN == 0
    sizes = [NT] * (C // NT)
    if C % NT:
        sizes.append(C % NT)
    return sizes


def build_moe_expert_kernel(C: int, repeat: int = 1, split_w: int = 8,
                            split_x: bool = True, split_y: bool = True,
                            pipe: bool = False, stagger: bool = False,
                            dve_elt: bool = True,
                            wb_all: bool = True) -> bacc.Bacc:
    """One-expert MLP kernel: yT = (relu(x@W1+b1)@W2 + b2) * w, chunk-major.

    DRAM inputs: xTb [nb, D, NT] (+ xTt [D, tail] if C%NT), wvec [1, C],
    ones [1, 128], w1 [D, D], b1 [D], w2 [D, D], b2 [D].
    Outputs: yTb [nb, D, NT] (+ yTt [D, tail]).
    `repeat` wraps the computation in a hardware loop (slope-based HW timing).
    """
    sizes = chunk_sizes(C)
    nb = sum(1 for s in sizes if s == NT)
    tail = C % NT

    nc = bacc.Bacc("TRN2", target_bir_lowering=False, debug=False,
                   num_devices=N_CORES)

    xTb = nc.dram_tensor("xTb", [nb, D, NT], F32R, kind="ExternalInput")
    wvec = nc.dram_tensor("wvec", [1, C], F32R, kind="ExternalInput")
    ones = nc.dram_tensor("ones", [1, 128], F32R, kind="ExternalInput")
    w1 = nc.dram_tensor("w1", [D, D], F32R, kind="ExternalInput")
    b1 = nc.dram_tensor("b1", [D], F32, kind="ExternalInput")
    w2 = nc.dram_tensor("w2", [D, D], F32R, kind="ExternalInput")
    b2 = nc.dram_tensor("b2", [D], F32, kind="ExternalInput")
    yTb = nc.dram_tensor("yTb", [nb, D, NT], F32, kind="ExternalOutput")
    if tail:
        xTt = nc.dram_tensor("xTt", [D, tail], F32R, kind="ExternalInput")
        yTt = nc.dram_tensor("yTt", [D, tail], F32, kind="ExternalOutput")

    # DRAM views: partition-dim-first tilings (chunk blocks are contiguous)
    xTb_v = xTb.ap().rearrange("n (kc kp) t -> n kp kc t", kc=KC)
    w1_v = w1.ap().rearrange("(kc kp) m -> kp kc m", kc=KC)      # [128, KC, D]
    w2_v = w2.ap().rearrange("(kc kp) m -> kp kc m", kc=KC)
    b1_v = b1.ap().rearrange("(mc mp) -> mp mc", mc=MC)          # [128, MC]
    b2_v = b2.ap().rearrange("(mc mp) -> mp mc", mc=MC)
    yTb_v = yTb.ap().rearrange("n (mc mp) t -> n mp mc t", mc=MC)
    if tail:
        xTt_v = xTt.ap().rearrange("(kc kp) t -> kp kc t", kc=KC)
        yTt_v = yTt.ap().rearrange("(mc mp) t -> mp mc t", mc=MC)

    def x_view(n):
        return xTb_v[n] if sizes[n] == NT else xTt_v

    def y_view(n):
        return yTb_v[n] if sizes[n] == NT else yTt_v

    with tile.TileContext(nc) as tc:
        with (
            tc.tile_pool(name="weights", bufs=1) as wpool,
            tc.tile_pool(name="consts", bufs=1) as cpool,
            tc.tile_pool(name="xin", bufs=3) as xpool,
            tc.tile_pool(name="hmid", bufs=2) as hpool,
            tc.tile_pool(name="yout", bufs=2) as ypool,
            tc.tile_pool(name="wbp", bufs=2) as wbpool,
            tc.tile_pool(name="ph", bufs=3, space="PSUM") as phpool,
            tc.tile_pool(name="py", bufs=3, space="PSUM") as pypool,
            tc.tile_pool(name="pw", bufs=2, space="PSUM") as pwpool,
        ):
            from contextlib import nullcontext
            loop_cm = (
                tc.For_i(0, repeat, 1,
                         hint_engines=(mybir.EngineType.PE,
                                       mybir.EngineType.Activation,
                                       mybir.EngineType.DVE,
                                       mybir.EngineType.SP),
                         staggered_reset=stagger)
                if repeat > 1 else nullcontext()
            )
            with loop_cm:
                # Per-kc split DMAs: the first matmul only waits for its own
                # 512KB weight slice + 256KB x slice instead of the whole
                # prologue (model: first MM 36.6us -> 5.1us).
                w1_sb = wpool.tile([128, KC, D], F32R, tag="w1")
                w2_sb = wpool.tile([128, KC, D], F32R, tag="w2")
                b1_sb = cpool.tile([128, MC], F32, tag="b1")
                b2_sb = cpool.tile([128, MC], F32, tag="b2")
                wv_sb = cpool.tile([1, C], F32R, tag="wv")
                on_sb = cpool.tile([1, 128], F32R, tag="ones")
                x0 = xpool.tile([128, KC, NT], F32R, tag="x")
                g = KC // split_w
                for i in range(split_w):
                    ks = slice(i * g, (i + 1) * g)
                    nc.sync.dma_start(w1_sb[:, ks, :], w1_v[:, ks, :])
                    if split_x:
                        for kc in range(i * g, (i + 1) * g):
                            nc.sync.dma_start(x0[:, kc, :sizes[0]],
                                              x_view(0)[:, kc, :])
                if not split_x:
                    nc.sync.dma_start(x0[:, :, :sizes[0]], x_view(0))
                nc.sync.dma_start(b1_sb[:], b1_v)
                nc.sync.dma_start(on_sb[:], ones.ap())
                nc.sync.dma_start(wv_sb[:], wvec.ap())

                offs = [sum(sizes[:i]) for i in range(len(sizes))]
                nchk = len(sizes)

                wb_full = None
                if wb_all:
                    wb_full = cpool.tile([128, C], F32, tag="wbf")
                    for n in range(nchk):
                        pwf = pwpool.tile([128, NT], F32, tag="pw")
                        nc.tensor.matmul(pwf[:, :sizes[n]], on_sb[:],
                                         wv_sb[:, offs[n]:offs[n] + sizes[n]],
                                         start=True, stop=True)
                        nc.vector.tensor_copy(
                            wb_full[:, offs[n]:offs[n] + sizes[n]],
                            pwf[:, :sizes[n]])

                def emit_x_dma(n, x_sb):
                    sz = sizes[n]
                    if split_x:
                        for kc in range(KC):
                            nc.sync.dma_start(x_sb[:, kc, :sz],
                                              x_view(n)[:, kc, :])
                    else:
                        nc.sync.dma_start(x_sb[:, :, :sz], x_view(n))

                def emit_wb(n):
                    # broadcast combine weights for chunk n: [128, sz]
                    if wb_all:
                        return wb_full[:, offs[n]:offs[n] + sizes[n]]
                    sz = sizes[n]
                    pw = pwpool.tile([128, NT], F32, tag="pw")
                    nc.tensor.matmul(pw[:, :sz], on_sb[:],
                                     wv_sb[:, offs[n]:offs[n] + sz],
                                     start=True, stop=True)
                    wb_sb = wbpool.tile([128, NT], F32, tag="wb")
                    nc.vector.tensor_copy(wb_sb[:, :sz], pw[:, :sz])
                    return wb_sb

                def emit_h_mc(n, mc, x_sb, h_sb):
                    sz = sizes[n]
                    ph = phpool.tile([128, NT], F32, tag="ph")
                    for kc in range(KC):
                        nc.tensor.matmul(
                            ph[:, :sz],
                            w1_sb[:, kc, bass.ts(mc, 128)],
                            x_sb[:, kc, :sz],
                            start=(kc == 0), stop=(kc == KC - 1),
                        )
                    # h = relu(ph + b1)
                    if dve_elt:
                        nc.vector.tensor_scalar(
                            h_sb[:, mc, :sz], ph[:, :sz],
                            b1_sb[:, mc:mc + 1], 0.0,
                            mybir.AluOpType.add, mybir.AluOpType.max,
                        )
                    else:
                        nc.scalar.activation(
                            h_sb[:, mc, :sz], ph[:, :sz],
                            mybir.ActivationFunctionType.Relu,
                            bias=b1_sb[:, mc:mc + 1],
                        )

                def emit_y_mc(n, mc, h_sb, y_sb, wb_sb):
                    sz = sizes[n]
                    py = pypool.tile([128, NT], F32, tag="py")
                    for kc in range(KC):
                        nc.tensor.matmul(
                            py[:, :sz],
                            w2_sb[:, kc, bass.ts(mc, 128)],
                            h_sb[:, kc, :sz],
                            start=(kc == 0), stop=(kc == KC - 1),
                        )
                    # y = (py + b2) * w
                    if dve_elt:
                        nc.vector.tensor_scalar(
                            y_sb[:, mc, :sz], py[:, :sz],
                            b2_sb[:, mc:mc + 1], None,
                            mybir.AluOpType.add,
                        )
                    else:
                        nc.scalar.activation(
                            y_sb[:, mc, :sz], py[:, :sz],
                            mybir.ActivationFunctionType.Identity,
                            bias=b2_sb[:, mc:mc + 1],
                        )
                    nc.vector.tensor_mul(
                        y_sb[:, mc, :sz], y_sb[:, mc, :sz],
                        wb_sb if wb_all else wb_sb[:, :sz],
                    )
                    if split_y:
                        nc.sync.dma_start(y_view(n)[:, mc, :],
                                          y_sb[:, mc, :sz])

                def emit_w2():
                    # w2 is first needed by chunk 0's y-phase (~27us in); keep
                    # its 4MB off the DMA engines while w1/x0 races the PE.
                    for i in range(split_w):
                        ks = slice(i * g, (i + 1) * g)
                        nc.sync.dma_start(w2_sb[:, ks, :], w2_v[:, ks, :])
                    nc.sync.dma_start(b2_sb[:], b2_v)

                if not pipe:
                    for n in range(nchk):
                        if n == 0:
                            x_sb = x0
                        else:
                            x_sb = xpool.tile([128, KC, NT], F32R, tag="x")
                            emit_x_dma(n, x_sb)
                        wb_sb = emit_wb(n)
                        h_sb = hpool.tile([128, KC, NT], F32R, tag="h")
                        for mc in range(MC):
                            emit_h_mc(n, mc, x_sb, h_sb)
                        if n == 0:
                            emit_w2()
                        y_sb = ypool.tile([128, MC, NT], F32, tag="y")
                        for mc in range(MC):
                            emit_y_mc(n, mc, h_sb, y_sb, wb_sb)
                        if not split_y:
                            nc.sync.dma_start(y_view(n), y_sb[:, :, :sizes[n]])
                else:
                    # software pipeline: stage s emits h-phase(s) interleaved
                    # with y-phase(s-1) at mc granularity, so the PE never
                    # waits on the relu tail of a chunk before starting the
                    # next chunk's first-layer matmuls.
                    emit_w2()
                    x_tiles = {0: x0}
                    h_tiles = {}
                    y_tiles = {}
                    wb_tiles = {0: emit_wb(0)}
                    for s in range(nchk + 1):
                        if s + 1 < nchk:  # prefetch x for next stage
                            xt = xpool.tile([128, KC, NT], F32R, tag="x")
                            emit_x_dma(s + 1, xt)
                            x_tiles[s + 1] = xt
                        if s < nchk:
                            h_tiles[s] = hpool.tile([128, KC, NT], F32R, tag="h", name=f"hs{s}")
                            if s + 1 < nchk:
                                wb_tiles[s + 1] = emit_wb(s + 1)
                        if s > 0:
                            y_tiles[s - 1] = ypool.tile([128, MC, NT], F32, tag="y", name=f"ys{s}")
                        for mc in range(MC):
                            if s < nchk:
                                emit_h_mc(s, mc, x_tiles[s], h_tiles[s])
                            if s > 0:
                                emit_y_mc(s - 1, mc, h_tiles[s - 1],
                                          y_tiles[s - 1], wb_tiles[s - 1])
                        if s > 0 and not split_y:
                            nc.sync.dma_start(y_view(s - 1),
                                              y_tiles[s - 1][:, :, :sizes[s - 1]])
                        x_tiles.pop(s - 1, None)

    nc.compile()
    return nc


_NC_CACHE: dict = {}


def _get_kernel(C: int, repeat: int = 1, **opts) -> bacc.Bacc:
    key = (C, repeat, tuple(sorted(opts.items())))
    if key not in _NC_CACHE:
        _NC_CACHE[key] = build_moe_expert_kernel(C, repeat, **opts)
    return _NC_CACHE[key]


def dispatch(x, W_gate, b_gate):
    """Host-side gate + top-2 dispatch plan. Returns (xf, ids, wts, C)."""
    xf = np.ascontiguousarray(np.asarray(x).reshape(-1, D), dtype=np.float32)
    scores = xf @ np.asarray(W_gate, np.float32) + np.asarray(b_gate, np.float32)
    # top-2 expert ids per token (order irrelevant: contributions are summed)
    top2 = np.argpartition(scores, N_EXPERTS - TOP_K, axis=1)[:, -TOP_K:]
    ids, wts = [], []
    for e in range(N_EXPERTS):
        tok = np.nonzero((top2 == e).any(axis=1))[0]
        ids.append(tok)
        wts.append(scores[tok, e])
    max_cnt = max(len(t) for t in ids)
    C = ((max_cnt + CGRAIN - 1) // CGRAIN) * CGRAIN
    return xf, ids, wts, C


def make_in_maps(parts, xf, ids, wts, C):
    """Build per-core input dicts (chunk-major xT blocks)."""
    W1, b1, W2, b2 = parts
    sizes = chunk_sizes(C)
    nb = sum(1 for s in sizes if s == NT)
    tail = C % NT
    in_maps = []
    for e in range(N_EXPERTS):
        cnt = len(ids[e])
        xTe = np.zeros((D, C), np.float32)
        xTe[:, :cnt] = xf[ids[e]].T
        xb = np.ascontiguousarray(
            xTe[:, :nb * NT].reshape(D, nb, NT).transpose(1, 0, 2))
        wv = np.zeros((1, C), np.float32)
        wv[0, :cnt] = wts[e]
        m = {
            "xTb": xb, "wvec": wv,
            "ones": np.ones((1, 128), np.float32),
            "w1": np.ascontiguousarray(W1[e]), "b1": b1[e],
            "w2": np.ascontiguousarray(W2[e]), "b2": b2[e],
        }
        if tail:
            m["xTt"] = np.ascontiguousarray(xTe[:, nb * NT:])
        in_maps.append(m)
    return in_maps


def kernel(x, W_gate, b_gate, W1, b1, W2, b2):
    xf, ids, wts, C = dispatch(x, W_gate, b_gate)
    nc = _get_kernel(C)

    W1 = np.asarray(W1, np.float32)
    W2 = np.asarray(W2, np.float32)
    b1 = np.asarray(b1, np.float32)
    b2 = np.asarray(b2, np.float32)
    in_maps = make_in_maps((W1, b1, W2, b2), xf, ids, wts, C)

    res = run_bass_kernel_spmd(nc, in_maps, core_ids=list(range(N_CORES)))

    sizes = chunk_sizes(C)
    nb = sum(1 for s in sizes if s == NT)
    tail = C % NT
    out = np.zeros((N_TOKENS, D), np.float32)
    for e in range(N_EXPERTS):
        cnt = len(ids[e])
        r = res.results[e]
        yTe = r["yTb"].transpose(1, 0, 2).reshape(D, nb * NT)
        if tail:
            yTe = np.concatenate([yTe, r["yTt"]], axis=1)
        out[ids[e]] += yTe.T[:cnt]
    return out.reshape(B, T, D)
